# revision 35
# baseline (speedup 1.0000x reference)
"""Trainium2 Bass kernel for nn_MultiHeadMALAAttention.

Sharding: 8 cores; core c handles batch b = c//2, token half h = c%2
(tokens [h*4096, (h+1)*4096) of N=8192).  Stats (kmean, vmean, kv_state)
need full-N reductions -> pairwise AllReduce between the two cores of a
batch, replica groups [[0,1],[2,3],[4,5],[6,7]].

Pipeline structure (v2 — restructured for overlap):
  phase K : k/v projections, elu(k)+1, rope(k), transposes + kv-gram,
            ksum/vsum accumulation.          (before the collective)
  AllReduce of [gram | ksum | vsum]  (133 KB, latency-bound ~25us)
  phase Q : q/o projections, elu(q)+1, rope(q) -> qs0.  Emitted after
            the collective start; no gpsimd use, so it runs *during*
            the collective.
  phase Z : small stats post-processing (zsc/zblk/kvblk/mcorr).
  phase C : z matmul, qa=(1+1/z)*qs0, attn+lepe+corr fused in PSUM,
            y=res*o, output projection, bf16 store.

On-device layout: channel-major ("CT", [chan partitions, token free])
throughout; token-major transient tiles (PE transpose) only for the
kv-gram contraction.  All matmuls bf16 (fp32 PSUM accum).
"""

import os
import sys

sys.path.insert(0, "/opt/trn_rl_repo")

import numpy as np
import ml_dtypes

B, N, DIM, H, HD = 4, 8192, 256, 8, 32
INTERNAL = H * HD  # 256
SCALE = HD ** -0.5
NCORES = 8
T = N // 2          # tokens per core
TH = T + 2          # with 1-token halo each side
CH = 512            # chunk tokens
NCH = T // CH       # chunks per core
KSC = SCALE / N     # kv_state scale (s^2)
P2 = 2 * CH         # paired free size 1024

BF16 = ml_dtypes.bfloat16


# ---------------------------------------------------------------- host prep

def _host_prep(x, sin, cos, W_qkvo, b_qkvo, W_lepe, b_lepe, W_proj, b_proj):
    """Build per-core input dicts (all device tensors)."""
    WT = W_qkvo.T.astype(np.float32)          # [DIM, 1024] = lhsT layout
    wq = WT[:, 0:256].astype(BF16)
    wkv = WT[:, 256:768].astype(BF16)          # k cols 0:256, v cols 256:512
    wo = WT[:, 768:1024].astype(BF16)
    wp = W_proj.T.astype(np.float32).astype(BF16)   # [DIM, 256] rhs layout
    wl = W_lepe[:, 0, :].astype(np.float32)    # [256, 3]

    # diag conv weights: block (tap j, tile m) = diag(wl[128m:128(m+1), j])
    dcw = np.zeros((128, 6, 128), np.float32)
    for j in range(3):
        for m in range(2):
            np.fill_diagonal(dcw[:, j * 2 + m, :], wl[128 * m:128 * (m + 1), j])
    dcw = dcw.reshape(128, 768).astype(BF16)

    # rotate-every-two matrix as lhsT: rot = R.T @ x ; R[k, m] = coeff of
    # chan k in rot-chan m:  rot[2i] = -x[2i+1], rot[2i+1] = x[2i]
    R = np.zeros((128, 128), np.float32)
    for i in range(64):
        R[2 * i + 1, 2 * i] = -1.0
        R[2 * i, 2 * i + 1] = 1.0
    R = R.astype(BF16)

    hmask = np.zeros((128, 128), np.float32)
    for hh in range(4):
        hmask[32 * hh:32 * (hh + 1), 32 * hh:32 * (hh + 1)] = 1.0
    hmk = (hmask * KSC).astype(BF16)           # pre-scaled mask for kvblk
    hmask = hmask.astype(BF16)

    ident16 = np.eye(128, dtype=np.float32).astype(BF16)

    # packed constants: one DMA each instead of many small ones
    wqop = np.concatenate([wq[0:128], wq[128:256], wo[0:128], wo[128:256],
                           wp[0:128], wp[128:256]], axis=1)      # [128, 1536]
    cpack = np.concatenate([R, ident16, hmask, hmk], axis=1)     # [128, 512]

    use_bias = bool(np.any(b_qkvo) or np.any(b_lepe) or np.any(b_proj))
    bqkvo = np.asarray(b_qkvo, np.float32).reshape(1, 1024).astype(BF16)
    blep = np.asarray(b_lepe, np.float32).reshape(1, 256).astype(BF16)
    bprj = np.asarray(b_proj, np.float32).reshape(1, 256).astype(BF16)

    xf = np.asarray(x, np.float32)
    sinf = np.asarray(sin, np.float32)
    cosf = np.asarray(cos, np.float32)

    per_core = []
    for c in range(NCORES):
        b = c // 2
        t0 = (c % 2) * T
        # x channel-major with halo [256, TH]
        xpad = np.zeros((TH, DIM), np.float32)
        lo, hi = t0 - 1, t0 + T + 1
        slo, shi = max(lo, 0), min(hi, N)
        xpad[slo - lo: slo - lo + (shi - slo)] = xf[b, slo:shi]
        xct = np.ascontiguousarray(xpad.T).astype(BF16)          # [256, TH]

        srep = np.tile(sinf[t0:t0 + T].T, (4, 1)).astype(BF16)   # [128, T]
        crep = np.tile(cosf[t0:t0 + T].T, (4, 1)).astype(BF16)   # [128, T]

        per_core.append({
            "xct": xct, "srep": np.ascontiguousarray(srep),
            "crep": np.ascontiguousarray(crep),
            "wqop": np.ascontiguousarray(wqop),
            "wkv": np.ascontiguousarray(wkv),
            "cpack": np.ascontiguousarray(cpack), "dcw": dcw,
            "bqkvo": bqkvo, "blep": blep, "bprj": bprj,
        })
    return per_core, use_bias


# ------------------------------------------------------------ device kernel

def _build_nc(use_bias: bool, nocc: bool = False):
    from concourse import bacc
    import concourse.mybir as mybir
    import concourse.tile as tile

    dt = mybir.dt
    AF = mybir.ActivationFunctionType
    OP = mybir.AluOpType

    nc = bacc.Bacc(None, target_bir_lowering=False)

    # ---- I/O
    xct_d = nc.dram_tensor("xct", [256, TH], dt.bfloat16, kind="ExternalInput")
    srep_d = nc.dram_tensor("srep", [128, T], dt.bfloat16, kind="ExternalInput")
    crep_d = nc.dram_tensor("crep", [128, T], dt.bfloat16, kind="ExternalInput")
    wqop_d = nc.dram_tensor("wqop", [128, 1536], dt.bfloat16,
                            kind="ExternalInput")
    wkv_d = nc.dram_tensor("wkv", [256, 512], dt.bfloat16, kind="ExternalInput")
    cpack_d = nc.dram_tensor("cpack", [128, 512], dt.bfloat16,
                             kind="ExternalInput")
    dcw_d = nc.dram_tensor("dcw", [128, 768], dt.bfloat16, kind="ExternalInput")
    bqkvo_d = nc.dram_tensor("bqkvo", [1, 1024], dt.bfloat16, kind="ExternalInput")
    blep_d = nc.dram_tensor("blep", [1, 256], dt.bfloat16, kind="ExternalInput")
    bprj_d = nc.dram_tensor("bprj", [1, 256], dt.bfloat16, kind="ExternalInput")
    out_d = nc.dram_tensor("out", [256, T], dt.bfloat16, kind="ExternalOutput")

    RG = [[0, 1], [2, 3], [4, 5], [6, 7]]

    with tile.TileContext(nc) as tc:
        with (
            tc.tile_pool(name="const", bufs=1) as const,
            tc.tile_pool(name="work", bufs=3) as work,
            tc.tile_pool(name="psum", bufs=2, space="PSUM") as ppool,
            tc.tile_pool(name="pacc", bufs=1, space="PSUM") as pacc,
            tc.tile_pool(name="dram", bufs=1, space="DRAM") as dpool,
        ):
            def load(tname, dten, shape, dtype=dt.bfloat16):
                t_ = const.tile(shape, dtype, tag=tname, name=tname)
                nc.sync.dma_start(out=t_, in_=dten[:, :])
                return t_

            # ---- input DMAs, critical-first: what chunk 0 of phase K needs
            # (wkv, xct piece 0, consts, sin/cos piece 0), then the rest.
            wkv = [const.tile([128, 512], dt.bfloat16, tag=f"wkv{k}", name=f"wkv{k}")
                   for k in range(2)]
            for k in range(2):
                nc.sync.dma_start(out=wkv[k],
                                  in_=wkv_d[128 * k:128 * (k + 1), :])
            xct = [const.tile([128, TH], dt.bfloat16, tag=f"xct{k}",
                              name=f"xct{k}") for k in range(2)]
            xcut = [0, 1025, 2049, 3073, TH]
            for k in range(2):
                nc.sync.dma_start(
                    out=xct[k][:, xcut[0]:xcut[1]],
                    in_=xct_d[128 * k:128 * (k + 1), xcut[0]:xcut[1]])
            cpk = const.tile([128, 512], dt.bfloat16, tag="cpk", name="cpk")
            nc.sync.dma_start(out=cpk, in_=cpack_d[:, :])
            rblk = cpk[:, 0:128]
            id16 = cpk[:, 128:256]
            hmask = cpk[:, 256:384]
            hmk = cpk[:, 384:512]
            srep = const.tile([128, T], dt.bfloat16, tag="srep", name="srep")
            crep = const.tile([128, T], dt.bfloat16, tag="crep", name="crep")
            nc.sync.dma_start(out=srep[:, 0:1024], in_=srep_d[:, 0:1024])
            nc.sync.dma_start(out=crep[:, 0:1024], in_=crep_d[:, 0:1024])
            # remaining pieces, interleaved in need order
            for p in range(1, 4):
                for k in range(2):
                    nc.sync.dma_start(
                        out=xct[k][:, xcut[p]:xcut[p + 1]],
                        in_=xct_d[128 * k:128 * (k + 1), xcut[p]:xcut[p + 1]])
                sl = slice(p * 1024, (p + 1) * 1024)
                nc.sync.dma_start(out=srep[:, sl], in_=srep_d[:, sl])
                nc.sync.dma_start(out=crep[:, sl], in_=crep_d[:, sl])
            wqop = const.tile([128, 1536], dt.bfloat16, tag="wqop", name="wqop")
            nc.sync.dma_start(out=wqop, in_=wqop_d[:, :])
            wq = [wqop[:, 256 * k:256 * (k + 1)] for k in range(2)]
            wo = [wqop[:, 512 + 256 * k:512 + 256 * (k + 1)] for k in range(2)]
            wp = [wqop[:, 1024 + 256 * k:1024 + 256 * (k + 1)] for k in range(2)]
            dcw = load("dcw", dcw_d, [128, 768])
            ones = const.tile([1, CH], dt.bfloat16, tag="ones", name="ones")
            nc.vector.memset(ones, 1.0)
            if use_bias:
                bqkvo = load("bqkvo", bqkvo_d, [1, 1024])
                blep = load("blep", blep_d, [1, 256])
                bprj = load("bprj", bprj_d, [1, 256])

            # persistent activations (paired layout: col c*1024 + j*512 + t)
            q1p = const.tile([128, 2 * T], dt.bfloat16, tag="q1p", name="q1p")
            qs0 = const.tile([128, 2 * T], dt.bfloat16, tag="qs0", name="qs0")
            o1p = const.tile([128, 2 * T], dt.bfloat16, tag="o1p", name="o1p")
            vT = [const.tile([128, TH], dt.bfloat16, tag=f"vT{j}", name=f"vT{j}")
                  for j in range(2)]
            kpart = const.tile([128, 16], dt.float32, tag="kpart", name="kpart")
            vpart = const.tile([128, 16], dt.float32, tag="vpart", name="vpart")
            stats = const.tile([128, 260], dt.float32, tag="stats", name="stats")
            stats2 = const.tile([128, 260], dt.float32, tag="stats2",
                                name="stats2")

            gram = pacc.tile([128, 256], dt.float32, tag="gram", name="gram")

            # =========================== phase K ===========================
            # k/v projections, elu(k)+1, rope(k), transposes + gram, stats.
            for c in range(NCH):
                xsl = [x[:, 1 + c * CH: 1 + (c + 1) * CH] for x in xct]
                ssl = srep[:, c * CH:(c + 1) * CH]
                csl = crep[:, c * CH:(c + 1) * CH]

                kps = ppool.tile([128, P2], dt.float32, tag="big", name="kps")
                vps = ppool.tile([128, P2], dt.float32, tag="big", name="vps")
                for j in range(2):
                    cols = slice(j * CH, (j + 1) * CH)
                    ksl = slice(128 * j, 128 * (j + 1))
                    vsl = slice(256 + 128 * j, 256 + 128 * (j + 1))
                    nc.tensor.matmul(kps[:, cols], wkv[0][:, ksl], xsl[0],
                                     start=True, stop=False)
                    nc.tensor.matmul(kps[:, cols], wkv[1][:, ksl], xsl[1],
                                     start=False, stop=not use_bias)
                    if use_bias:
                        nc.tensor.matmul(kps[:, cols],
                                         bqkvo[:, 256 + 128 * j:256 + 128 * (j + 1)],
                                         ones, start=False, stop=True)
                    nc.tensor.matmul(vps[:, cols], wkv[0][:, vsl], xsl[0],
                                     start=True, stop=False)
                    nc.tensor.matmul(vps[:, cols], wkv[1][:, vsl], xsl[1],
                                     start=False, stop=not use_bias)
                    if use_bias:
                        nc.tensor.matmul(vps[:, cols],
                                         bqkvo[:, 512 + 128 * j:512 + 128 * (j + 1)],
                                         ones, start=False, stop=True)

                # elu(k)+1 : exp(min(k,0)) = min(exp(k),1);  +relu(k)
                ek = work.tile([128, P2], dt.bfloat16, tag="ek", name="ek")
                nc.scalar.activation(ek, kps, AF.Exp)
                mnk = work.tile([128, P2], dt.bfloat16, tag="mnk", name="mnk")
                nc.vector.tensor_scalar_min(mnk, ek, 1.0)
                k1t = work.tile([128, P2], dt.bfloat16, tag="k1t", name="k1t")
                for j in range(2):
                    cols = slice(j * CH, (j + 1) * CH)
                    nc.vector.scalar_tensor_tensor(
                        out=k1t[:, cols], in0=kps[:, cols], scalar=0.0,
                        in1=mnk[:, cols], op0=OP.max, op1=OP.add,
                        accum_out=kpart[:, 8 * j + c: 8 * j + c + 1])
                    # v evac -> vT channel-major (vsum rides ACT accum)
                    nc.scalar.activation(
                        vT[j][:, 1 + c * CH: 1 + (c + 1) * CH],
                        vps[:, j * CH:(j + 1) * CH], AF.Copy,
                        accum_out=vpart[:, 8 * j + c: 8 * j + c + 1])

                # rope(k): ks = k1t*cos + (R.T@k1t)*sin
                m1 = work.tile([128, P2], dt.bfloat16, tag="m1", name="m1")
                nc.gpsimd.tensor_mul(
                    m1[:, :].rearrange("p (r t) -> p r t", r=2),
                    k1t[:, :].rearrange("p (r t) -> p r t", r=2),
                    csl.unsqueeze(1).to_broadcast((128, 2, CH)))
                m2 = work.tile([128, P2], dt.bfloat16, tag="m2", name="m2")
                for j in range(2):
                    cols = slice(j * CH, (j + 1) * CH)
                    rkp = ppool.tile([128, CH], dt.float32, tag="tp", bufs=3,
                                     name="rkp")
                    nc.tensor.matmul(rkp, rblk, k1t[:, cols],
                                     start=True, stop=True)
                    nc.vector.tensor_mul(m2[:, cols], rkp, ssl)
                ks = work.tile([128, P2], dt.bfloat16, tag="ks", name="ks")
                nc.gpsimd.tensor_add(ks, m1, m2)

                # transposes to token-major; kv gram accumulation
                for s in range(4):
                    ktp = ppool.tile([128, CH], dt.bfloat16, tag="tp", bufs=3,
                                     name="ktp")
                    nc.tensor.transpose(ktp[:, 0:128],
                                        ks[:, s * 128:(s + 1) * 128], id16)
                    nc.tensor.transpose(ktp[:, 128:256],
                                        ks[:, CH + s * 128:CH + (s + 1) * 128],
                                        id16)
                    vcol = 1 + c * CH + s * 128
                    nc.tensor.transpose(ktp[:, 256:384],
                                        vT[0][:, vcol:vcol + 128], id16)
                    nc.tensor.transpose(ktp[:, 384:512],
                                        vT[1][:, vcol:vcol + 128], id16)
                    kvtok = work.tile([128, CH], dt.bfloat16, tag="kvtok",
                                      name="kvtok")
                    if s % 2 == 1:
                        nc.vector.tensor_copy(kvtok, ktp)
                    else:
                        nc.scalar.activation(kvtok, ktp, AF.Copy)
                    first = (c == 0 and s == 0)
                    last = (c == NCH - 1 and s == 3)
                    nc.tensor.matmul(gram[:, 0:128], kvtok[:, 0:128],
                                     kvtok[:, 256:384], start=first, stop=False)
                    nc.tensor.matmul(gram[:, 128:256], kvtok[:, 128:256],
                                     kvtok[:, 384:512], start=False, stop=last)

            # ---- halo v columns (tokens t0-1 and t0+T) for the conv
            vhp = ppool.tile([128, CH], dt.float32, tag="tp", bufs=3, name="vhp")
            for j in range(2):
                vsl = slice(256 + 128 * j, 256 + 128 * (j + 1))
                cl = slice(j * 4, j * 4 + 1)
                cr = slice(j * 4 + 2, j * 4 + 3)
                nc.tensor.matmul(vhp[:, cl], wkv[0][:, vsl], xct[0][:, 0:1],
                                 start=(j == 0), stop=False)
                nc.tensor.matmul(vhp[:, cl], wkv[1][:, vsl], xct[1][:, 0:1],
                                 start=False, stop=False)
                nc.tensor.matmul(vhp[:, cr], wkv[0][:, vsl], xct[0][:, TH - 1:TH],
                                 start=False, stop=False)
                nc.tensor.matmul(vhp[:, cr], wkv[1][:, vsl], xct[1][:, TH - 1:TH],
                                 start=False, stop=(j == 1))
            for j in range(2):
                nc.scalar.activation(vT[j][:, 0:1], vhp[:, j * 4:j * 4 + 1],
                                     AF.Copy)
                nc.scalar.activation(vT[j][:, TH - 1:TH],
                                     vhp[:, j * 4 + 2:j * 4 + 3], AF.Copy)

            # ======================= stats + allreduce =====================
            nc.vector.tensor_scalar_mul(stats[:, 0:256], gram, 1.0)
            nc.vector.tensor_reduce(stats[:, 256:257], kpart[:, 0:8],
                                    axis=mybir.AxisListType.X, op=OP.add)
            nc.vector.tensor_reduce(stats[:, 257:258], kpart[:, 8:16],
                                    axis=mybir.AxisListType.X, op=OP.add)
            nc.vector.tensor_reduce(stats[:, 258:259], vpart[:, 0:8],
                                    axis=mybir.AxisListType.X, op=OP.add)
            nc.vector.tensor_reduce(stats[:, 259:260], vpart[:, 8:16],
                                    axis=mybir.AxisListType.X, op=OP.add)

            if nocc:
                nc.vector.tensor_scalar_mul(stats2, stats, 1.0)
            else:
                ccin = dpool.tile([128, 260], dt.float32, tag="ccin",
                                  name="ccin")
                ccout = dpool.tile([128, 260], dt.float32, tag="ccout",
                                   name="ccout")
                nc.gpsimd.dma_start(out=ccin[:, :], in_=stats)
                nc.gpsimd.collective_compute(
                    "AllReduce", OP.add, replica_groups=RG,
                    ins=[ccin[:, :]], outs=[ccout[:, :]])
                # dma-back on sync so the gpsimd queue stays free during
                # phase Q (dependency on the collective is tracked via ccout)
                nc.sync.dma_start(out=stats2, in_=ccout[:, :])

            # =========================== phase Q ===========================
            # q/o projections + rope(q).  Independent of the collective ->
            # overlaps it.  No gpsimd here (its queue waits on the AR).
            for c in range(NCH):
                xsl = [x[:, 1 + c * CH: 1 + (c + 1) * CH] for x in xct]
                psl = slice(c * P2, (c + 1) * P2)
                ssl = srep[:, c * CH:(c + 1) * CH]
                csl = crep[:, c * CH:(c + 1) * CH]

                qps = ppool.tile([128, P2], dt.float32, tag="big", name="qps")
                ops_ = ppool.tile([128, P2], dt.float32, tag="big", name="ops_")
                for j in range(2):
                    cols = slice(j * CH, (j + 1) * CH)
                    msl = slice(128 * j, 128 * (j + 1))
                    nc.tensor.matmul(qps[:, cols], wq[0][:, msl], xsl[0],
                                     start=True, stop=False)
                    nc.tensor.matmul(qps[:, cols], wq[1][:, msl], xsl[1],
                                     start=False, stop=not use_bias)
                    if use_bias:
                        nc.tensor.matmul(qps[:, cols], bqkvo[:, msl], ones,
                                         start=False, stop=True)
                    nc.tensor.matmul(ops_[:, cols], wo[0][:, msl], xsl[0],
                                     start=True, stop=False)
                    nc.tensor.matmul(ops_[:, cols], wo[1][:, msl], xsl[1],
                                     start=False, stop=not use_bias)
                    if use_bias:
                        nc.tensor.matmul(ops_[:, cols],
                                         bqkvo[:, 768 + 128 * j:768 + 128 * (j + 1)],
                                         ones, start=False, stop=True)

                # elu(q)+1 -> q1p
                eq = work.tile([128, P2], dt.bfloat16, tag="ek", name="eq")
                nc.scalar.activation(eq, qps, AF.Exp)
                mnq = work.tile([128, P2], dt.bfloat16, tag="mnk", name="mnq")
                nc.vector.tensor_scalar_min(mnq, eq, 1.0)
                nc.vector.scalar_tensor_tensor(
                    out=q1p[:, psl], in0=qps, scalar=0.0, in1=mnq,
                    op0=OP.max, op1=OP.add)

                # o evac
                nc.scalar.activation(o1p[:, psl], ops_, AF.Copy)

                # rope(q) -> qs0  (all on DVE; gpsimd queue is busy w/ AR)
                m1 = work.tile([128, P2], dt.bfloat16, tag="m1", name="m1q")
                nc.vector.tensor_mul(
                    m1[:, :].rearrange("p (r t) -> p r t", r=2),
                    q1p[:, psl].rearrange("p (r t) -> p r t", r=2),
                    csl.unsqueeze(1).to_broadcast((128, 2, CH)))
                m2 = work.tile([128, P2], dt.bfloat16, tag="m2", name="m2q")
                for j in range(2):
                    cols = slice(j * CH, (j + 1) * CH)
                    rqp = ppool.tile([128, CH], dt.float32, tag="tp", bufs=3,
                                     name="rqp")
                    nc.tensor.matmul(
                        rqp, rblk,
                        q1p[:, c * P2 + j * CH: c * P2 + (j + 1) * CH],
                        start=True, stop=True)
                    nc.vector.tensor_mul(m2[:, cols], rqp, ssl)
                nc.vector.tensor_add(qs0[:, psl], m1, m2)

            # =========================== phase Z ===========================
            zsc = const.tile([128, 2], dt.float32, tag="zsc", name="zsc")
            nc.scalar.activation(zsc, stats2[:, 256:258], AF.Copy,
                                 scale=float(SCALE / N))
            zblk = []
            kvblk = []
            mcorr = []
            for j in range(2):
                zb = const.tile([128, 128], dt.bfloat16, tag=f"zblk{j}",
                                name=f"zblk{j}")
                nc.vector.tensor_tensor(
                    zb, zsc[:, j:j + 1].to_broadcast((128, 128)), hmask,
                    OP.mult)
                zblk.append(zb)
                kvb = const.tile([128, 128], dt.bfloat16, tag=f"kvb{j}",
                                 name=f"kvb{j}")
                nc.vector.tensor_tensor(
                    kvb, stats2[:, 128 * j:128 * (j + 1)], hmk, OP.mult)
                kvblk.append(kvb)

            # vmean row: cast (scaled by -1/N) -> transpose -> broadcast
            vrin = const.tile([128, 2], dt.bfloat16, tag="vrin", name="vrin")
            nc.scalar.activation(vrin, stats2[:, 258:260], AF.Copy,
                                 scale=float(-1.0 / N))
            vtp = ppool.tile([128, CH], dt.bfloat16, tag="tp", bufs=3,
                             name="vtp")
            for j in range(2):
                nc.tensor.transpose(vtp[0:1, 128 * j:128 * (j + 1)],
                                    vrin[:, j:j + 1], id16)
            vrow = const.tile([1, 256], dt.bfloat16, tag="vrow", name="vrow")
            nc.scalar.activation(vrow, vtp[0:1, 0:256], AF.Copy)
            for j in range(2):
                # broadcast vrow across partitions via a rank-1 matmul
                vrbp = ppool.tile([128, CH], dt.float32, tag="tp", bufs=3,
                                  name="vrbp")
                nc.tensor.matmul(vrbp[:, 0:128], ones[:, 0:128],
                                 vrow[:, 128 * j:128 * (j + 1)],
                                 start=True, stop=True)
                mc = const.tile([128, 128], dt.bfloat16, tag=f"mc{j}",
                                name=f"mc{j}")
                nc.vector.tensor_tensor(mc, zblk[j], vrbp[:, 0:128], OP.mult)
                mcorr.append(mc)

            # =========================== phase C ===========================
            # software-pipelined: chunk c+1's z/recip/qa (DVE) runs under
            # chunk c's y/outproj (PE/ACT).
            def zrq(c):
                rz = work.tile([128, P2], dt.float32, tag="rz", name="rz")
                qa = work.tile([128, P2], dt.bfloat16, tag="qa", name="qa")
                for j in range(2):
                    cols = slice(j * CH, (j + 1) * CH)
                    qsl = slice(c * P2 + j * CH, c * P2 + (j + 1) * CH)
                    zps = ppool.tile([128, CH], dt.float32, tag="tp", bufs=3,
                                     name="zps")
                    nc.tensor.matmul(zps, zblk[j], q1p[:, qsl],
                                     start=True, stop=True)
                    nc.vector.reciprocal_approx_fast(out=rz[:, cols], in_=zps)
                    nc.vector.scalar_tensor_tensor(
                        out=qa[:, cols], in0=rz[:, cols], scalar=1.0,
                        in1=qs0[:, qsl], op0=OP.add, op1=OP.mult)
                return qa

            qa_c = zrq(0)
            for c in range(NCH):
                psl = slice(c * P2, (c + 1) * P2)
                rps = ppool.tile([128, P2], dt.float32, tag="big", name="rps")
                for j in range(2):
                    cols = slice(j * CH, (j + 1) * CH)
                    qsl = slice(c * P2 + j * CH, c * P2 + (j + 1) * CH)
                    nc.tensor.matmul(rps[:, cols], kvblk[j], qa_c[:, cols],
                                     start=True, stop=False)
                    nc.tensor.matmul(rps[:, cols], mcorr[j], q1p[:, qsl],
                                     start=False, stop=False)
                    for tap in range(3):
                        lastmm = (tap == 2 and not use_bias)
                        nc.tensor.matmul(
                            rps[:, cols],
                            dcw[:, (tap * 2 + j) * 128:(tap * 2 + j + 1) * 128],
                            vT[j][:, c * CH + tap: c * CH + tap + CH],
                            start=False, stop=lastmm)
                    if use_bias:
                        nc.tensor.matmul(rps[:, cols],
                                         blep[:, 128 * j:128 * (j + 1)],
                                         ones, start=False, stop=True)

                if c + 1 < NCH:
                    qa_n = zrq(c + 1)

                y = work.tile([128, P2], dt.bfloat16, tag="y", name="y")
                nc.vector.tensor_mul(y, rps, o1p[:, psl])

                # output projection, channel-major: outc[oc, t] (per oc-half)
                for h in range(2):
                    hsl = slice(128 * h, 128 * (h + 1))
                    outc = ppool.tile([128, CH], dt.float32, tag="tp", bufs=3,
                                      name="outc")
                    nc.tensor.matmul(outc, wp[0][:, hsl], y[:, 0:CH],
                                     start=True, stop=False)
                    nc.tensor.matmul(outc, wp[1][:, hsl], y[:, CH:P2],
                                     start=False, stop=not use_bias)
                    if use_bias:
                        nc.tensor.matmul(outc, bprj[:, hsl], ones,
                                         start=False, stop=True)
                    outsb = work.tile([128, CH], dt.bfloat16, tag="outsb",
                                      name="outsb")
                    nc.scalar.activation(outsb, outc, AF.Copy)
                    nc.sync.dma_start(
                        out=out_d[128 * h:128 * (h + 1), c * CH:(c + 1) * CH],
                        in_=outsb)
                if c + 1 < NCH:
                    qa_c = qa_n

    nc.compile()
    return nc


_NC_CACHE = {}


def _get_nc(use_bias: bool):
    nocc = bool(os.environ.get("KERNEL_NOCC"))
    key = (use_bias, nocc)
    if key not in _NC_CACHE:
        _NC_CACHE[key] = _build_nc(use_bias, nocc)
    return _NC_CACHE[key]


def kernel(x, sin, cos, W_qkvo, b_qkvo, W_lepe, b_lepe, W_proj, b_proj):
    from concourse.bass_utils import run_bass_kernel_spmd

    per_core, use_bias = _host_prep(x, sin, cos, W_qkvo, b_qkvo, W_lepe,
                                    b_lepe, W_proj, b_proj)
    nc = _get_nc(use_bias)
    # keep only the inputs that survived DCE in the compiled program
    import concourse.mybir as mybir
    expected = set()
    for alloc in nc.m.functions[0].allocations:
        if isinstance(alloc, mybir.MemoryLocationSet) and alloc.kind == "ExternalInput":
            expected.add(alloc.memorylocations[0].name)
    per_core = [{k: v for k, v in m.items() if k in expected} for m in per_core]
    res = run_bass_kernel_spmd(nc, per_core, core_ids=list(range(NCORES)),
                               trace=bool(os.environ.get("KERNEL_TRACE")))
    if os.environ.get("KERNEL_TRACE"):
        kernel.last_exec_time_ns = res.exec_time_ns
        kernel.last_results = res
    full = np.zeros((B, N, INTERNAL), np.float32)
    for c in range(NCORES):
        b = c // 2
        t0 = (c % 2) * T
        full[b, t0:t0 + T] = res.results[c]["out"].astype(np.float32).T
    return full


# ---------------------------------------------------------- numpy reference
# A numpy emulation of the device pipeline (fp32), used to validate the
# decomposition (run with KERNEL_SELFTEST=1).

def _numpy_pipeline(per_core_inputs, skip_pair=False):
    outs = []
    cores = []
    for c in range(NCORES):
        d = per_core_inputs[c]
        xct = d["xct"].astype(np.float32)          # [256, TH]
        srep = d["srep"].astype(np.float32)
        crep = d["crep"].astype(np.float32)
        wqop = d["wqop"].astype(np.float32)
        wq = np.concatenate([wqop[:, 0:256], wqop[:, 256:512]], axis=0)
        wo = np.concatenate([wqop[:, 512:768], wqop[:, 768:1024]], axis=0)
        wp = np.concatenate([wqop[:, 1024:1280], wqop[:, 1280:1536]], axis=0)
        wkv = d["wkv"].astype(np.float32)
        dcw = d["dcw"].astype(np.float32).reshape(128, 6, 128)
        cpack = d["cpack"].astype(np.float32)
        R = cpack[:, 0:128]
        hmask = cpack[:, 256:384]

        x_in = xct[:, 1:T + 1]                     # [256, T]
        qT = wq.T @ x_in                           # [256, T]
        kT = wkv[:, 0:256].T @ x_in
        vT_m = wkv[:, 256:512].T @ x_in
        oT = wo.T @ x_in
        # halo v cols
        vhl = wkv[:, 256:512].T @ xct[:, 0:1]
        vhr = wkv[:, 256:512].T @ xct[:, TH - 1:TH]
        vT = np.concatenate([vhl, vT_m, vhr], axis=1)      # [256, TH]

        def elu1(t):
            return np.minimum(np.exp(t), 1.0) + np.maximum(t, 0.0)

        q1 = elu1(qT)
        k1 = elu1(kT)

        # K rope (per chan-tile with R)
        ks = np.zeros_like(k1)
        qs = np.zeros_like(q1)
        for j in range(2):
            blk = k1[128 * j:128 * (j + 1)]
            ks[128 * j:128 * (j + 1)] = blk * crep + (R.T @ blk) * srep
            qb = q1[128 * j:128 * (j + 1)]
            qs[128 * j:128 * (j + 1)] = qb * crep + (R.T @ qb) * srep

        # kv gram per tile: ks_j^T tokens x v_j
        gram = np.zeros((128, 256), np.float32)
        for j in range(2):
            gram[:, 128 * j:128 * (j + 1)] = (
                ks[128 * j:128 * (j + 1)] @ vT[128 * j:128 * (j + 1), 1:T + 1].T)
        ksum = k1.sum(axis=1)                      # [256]
        vsum = vT[:, 1:T + 1].sum(axis=1)
        cores.append(dict(d=d, q1=q1, qs=qs, oT=oT, vT=vT, gram=gram,
                          ksum=ksum, vsum=vsum, R=R, hmask=hmask, dcw=dcw,
                          wp=wp))

    for pair in range(4):
        a, b2 = cores[2 * pair], cores[2 * pair + 1]
        if skip_pair:
            for cc in (a, b2):
                cc["gram_r"], cc["ksum_r"], cc["vsum_r"] = (
                    cc["gram"], cc["ksum"], cc["vsum"])
            continue
        gram = a["gram"] + b2["gram"]
        ksum = a["ksum"] + b2["ksum"]
        vsum = a["vsum"] + b2["vsum"]
        for cc in (a, b2):
            cc["gram_r"], cc["ksum_r"], cc["vsum_r"] = gram, ksum, vsum

    for c in range(NCORES):
        st = cores[c]
        q1, qs, oT, vT = st["q1"], st["qs"], st["oT"], st["vT"]
        hmask, dcw, wp = st["hmask"], st["dcw"], st["wp"]
        gram, ksum, vsum = st["gram_r"], st["ksum_r"], st["vsum_r"]

        kmean = ksum / N
        vmean = vsum / N
        res = np.zeros((256, T), np.float32)
        for j in range(2):
            sl = slice(128 * j, 128 * (j + 1))
            zsc = SCALE * kmean[sl]                          # [128]
            zblk = (zsc[:, None] * hmask)                    # [128,128]
            zrep = zblk.T @ q1[sl]                           # [128, T]
            r = 1.0 / zrep
            qa = qs[sl] * (1.0 + r)
            kvblk = KSC * gram[:, 128 * j:128 * (j + 1)] * hmask
            mcorr = -(zsc[:, None]) * vmean[sl][None, :] * hmask
            lepe = np.zeros((128, T), np.float32)
            for tap in range(3):
                dw = dcw[:, tap * 2 + j, :]
                lepe += dw.T @ vT[sl, tap:tap + T]
            res[sl] = (kvblk.T @ qa + mcorr.T @ q1[sl] + lepe)
        y = res * oT
        out = y.T @ wp
        outs.append(out.astype(np.float32))

    # unshard
    full = np.zeros((B, N, 256), np.float32)
    for c in range(NCORES):
        b = c // 2
        t0 = (c % 2) * T
        full[b, t0:t0 + T] = outs[c]
    return full


if __name__ == "__main__" and os.environ.get("KERNEL_BUILD"):
    nc = _build_nc(False)
    import tempfile
    from concourse.bass_utils import compile_bass_kernel
    print("NEFF:", compile_bass_kernel(nc, tempfile.mkdtemp()))

if __name__ == "__main__" and os.environ.get("KERNEL_SELFTEST"):
    sys.path.insert(0, os.path.dirname(os.path.abspath(__file__)))
    import reference
    inputs = reference.setup_inputs()
    inputs = {k: np.asarray(v) for k, v in inputs.items()}
    expected = np.asarray(reference.reference(**inputs))
    per_core, use_bias = _host_prep(**inputs)
    got = _numpy_pipeline(per_core)
    err = np.abs(got - expected)
    rel = np.linalg.norm(got - expected) / np.linalg.norm(expected)
    print("selftest rel err:", rel, "max abs:", err.max())


# revision 36
# speedup vs baseline: 1.1116x; 1.1116x over previous
"""Trainium2 Bass kernel for nn_MultiHeadMALAAttention.

Sharding: 8 cores; core c handles batch b = c//2, token half h = c%2
(tokens [h*4096, (h+1)*4096) of N=8192).  Stats (kmean, vmean, kv_state)
need full-N reductions -> pairwise AllReduce between the two cores of a
batch, replica groups [[0,1],[2,3],[4,5],[6,7]].

Pipeline structure (v2 — restructured for overlap):
  phase K : k/v projections, elu(k)+1, rope(k), transposes + kv-gram,
            ksum/vsum accumulation.          (before the collective)
  AllReduce of [gram | ksum | vsum]  (133 KB, latency-bound ~25us)
  phase Q : q/o projections, elu(q)+1, rope(q) -> qs0.  Emitted after
            the collective start; no gpsimd use, so it runs *during*
            the collective.
  phase Z : small stats post-processing (zsc/zblk/kvblk/mcorr).
  phase C : z matmul, qa=(1+1/z)*qs0, attn+lepe+corr fused in PSUM,
            y=res*o, output projection, bf16 store.

On-device layout: channel-major ("CT", [chan partitions, token free])
throughout; token-major transient tiles (PE transpose) only for the
kv-gram contraction.  All matmuls bf16 (fp32 PSUM accum).
"""

import os
import sys

sys.path.insert(0, "/opt/trn_rl_repo")

import numpy as np
import ml_dtypes

B, N, DIM, H, HD = 4, 8192, 256, 8, 32
INTERNAL = H * HD  # 256
SCALE = HD ** -0.5
NCORES = 8
T = N // 2          # tokens per core
TH = T + 2          # with 1-token halo each side
CH = 512            # chunk tokens
NCH = T // CH       # chunks per core
KSC = SCALE / N     # kv_state scale (s^2)
P2 = 2 * CH         # paired free size 1024

BF16 = ml_dtypes.bfloat16


# ---------------------------------------------------------------- host prep

def _host_prep(x, sin, cos, W_qkvo, b_qkvo, W_lepe, b_lepe, W_proj, b_proj):
    """Build per-core input dicts (all device tensors)."""
    WT = W_qkvo.T.astype(np.float32)          # [DIM, 1024] = lhsT layout
    wq = WT[:, 0:256].astype(BF16)
    wkv = WT[:, 256:768].astype(BF16)          # k cols 0:256, v cols 256:512
    wo = WT[:, 768:1024].astype(BF16)
    wp = W_proj.T.astype(np.float32).astype(BF16)   # [DIM, 256] rhs layout
    wl = W_lepe[:, 0, :].astype(np.float32)    # [256, 3]

    # diag conv weights: block (tap j, tile m) = diag(wl[128m:128(m+1), j])
    dcw = np.zeros((128, 6, 128), np.float32)
    for j in range(3):
        for m in range(2):
            np.fill_diagonal(dcw[:, j * 2 + m, :], wl[128 * m:128 * (m + 1), j])
    dcw = dcw.reshape(128, 768).astype(BF16)

    # rotate-every-two matrix as lhsT: rot = R.T @ x ; R[k, m] = coeff of
    # chan k in rot-chan m:  rot[2i] = -x[2i+1], rot[2i+1] = x[2i]
    R = np.zeros((128, 128), np.float32)
    for i in range(64):
        R[2 * i + 1, 2 * i] = -1.0
        R[2 * i, 2 * i + 1] = 1.0
    R = R.astype(BF16)

    hmask = np.zeros((128, 128), np.float32)
    for hh in range(4):
        hmask[32 * hh:32 * (hh + 1), 32 * hh:32 * (hh + 1)] = 1.0
    hmk = (hmask * KSC).astype(BF16)           # pre-scaled mask for kvblk
    hmask = hmask.astype(BF16)

    ident16 = np.eye(128, dtype=np.float32).astype(BF16)

    # packed constants: one DMA each instead of many small ones
    wqop = np.concatenate([wq[0:128], wq[128:256], wo[0:128], wo[128:256],
                           wp[0:128], wp[128:256]], axis=1)      # [128, 1536]
    cpack = np.concatenate([R, ident16, hmask, hmk], axis=1)     # [128, 512]

    use_bias = bool(np.any(b_qkvo) or np.any(b_lepe) or np.any(b_proj))
    bqkvo = np.asarray(b_qkvo, np.float32).reshape(1, 1024).astype(BF16)
    blep = np.asarray(b_lepe, np.float32).reshape(1, 256).astype(BF16)
    bprj = np.asarray(b_proj, np.float32).reshape(1, 256).astype(BF16)

    xf = np.asarray(x, np.float32)
    sinf = np.asarray(sin, np.float32)
    cosf = np.asarray(cos, np.float32)

    per_core = []
    for c in range(NCORES):
        b = c // 2
        t0 = (c % 2) * T
        # x channel-major with halo [256, TH]
        xpad = np.zeros((TH, DIM), np.float32)
        lo, hi = t0 - 1, t0 + T + 1
        slo, shi = max(lo, 0), min(hi, N)
        xpad[slo - lo: slo - lo + (shi - slo)] = xf[b, slo:shi]
        xct = np.ascontiguousarray(xpad.T).astype(BF16)          # [256, TH]

        srep = np.tile(sinf[t0:t0 + T].T, (4, 1)).astype(BF16)   # [128, T]
        crep = np.tile(cosf[t0:t0 + T].T, (4, 1)).astype(BF16)   # [128, T]

        per_core.append({
            "xct": xct, "srep": np.ascontiguousarray(srep),
            "crep": np.ascontiguousarray(crep),
            "wqop": np.ascontiguousarray(wqop),
            "wkv": np.ascontiguousarray(wkv),
            "cpack": np.ascontiguousarray(cpack), "dcw": dcw,
            "bqkvo": bqkvo, "blep": blep, "bprj": bprj,
        })
    return per_core, use_bias


# ------------------------------------------------------------ device kernel

def _build_nc(use_bias: bool, nocc: bool = False):
    from concourse import bacc
    import concourse.mybir as mybir
    import concourse.tile as tile

    dt = mybir.dt
    AF = mybir.ActivationFunctionType
    OP = mybir.AluOpType

    nc = bacc.Bacc(None, target_bir_lowering=False)

    # ---- I/O
    xct_d = nc.dram_tensor("xct", [256, TH], dt.bfloat16, kind="ExternalInput")
    srep_d = nc.dram_tensor("srep", [128, T], dt.bfloat16, kind="ExternalInput")
    crep_d = nc.dram_tensor("crep", [128, T], dt.bfloat16, kind="ExternalInput")
    wqop_d = nc.dram_tensor("wqop", [128, 1536], dt.bfloat16,
                            kind="ExternalInput")
    wkv_d = nc.dram_tensor("wkv", [256, 512], dt.bfloat16, kind="ExternalInput")
    cpack_d = nc.dram_tensor("cpack", [128, 512], dt.bfloat16,
                             kind="ExternalInput")
    dcw_d = nc.dram_tensor("dcw", [128, 768], dt.bfloat16, kind="ExternalInput")
    bqkvo_d = nc.dram_tensor("bqkvo", [1, 1024], dt.bfloat16, kind="ExternalInput")
    blep_d = nc.dram_tensor("blep", [1, 256], dt.bfloat16, kind="ExternalInput")
    bprj_d = nc.dram_tensor("bprj", [1, 256], dt.bfloat16, kind="ExternalInput")
    out_d = nc.dram_tensor("out", [256, T], dt.bfloat16, kind="ExternalOutput")

    RG = [[0, 1], [2, 3], [4, 5], [6, 7]]

    with tile.TileContext(nc) as tc:
        with (
            tc.tile_pool(name="const", bufs=1) as const,
            tc.tile_pool(name="work", bufs=3) as work,
            tc.tile_pool(name="psum", bufs=2, space="PSUM") as ppool,
            tc.tile_pool(name="pacc", bufs=1, space="PSUM") as pacc,
            tc.tile_pool(name="dram", bufs=1, space="DRAM") as dpool,
        ):
            def load(tname, dten, shape, dtype=dt.bfloat16):
                t_ = const.tile(shape, dtype, tag=tname, name=tname)
                nc.sync.dma_start(out=t_, in_=dten[:, :])
                return t_

            # ---- input DMAs, critical-first: what chunk 0 of phase K needs
            # (wkv, xct piece 0, consts, sin/cos piece 0), then the rest.
            wkv = [const.tile([128, 512], dt.bfloat16, tag=f"wkv{k}", name=f"wkv{k}")
                   for k in range(2)]
            for k in range(2):
                nc.sync.dma_start(out=wkv[k],
                                  in_=wkv_d[128 * k:128 * (k + 1), :])
            xct = [const.tile([128, TH], dt.bfloat16, tag=f"xct{k}",
                              name=f"xct{k}") for k in range(2)]
            xcut = [0, 1025, 2049, 3073, TH]
            for k in range(2):
                nc.sync.dma_start(
                    out=xct[k][:, xcut[0]:xcut[1]],
                    in_=xct_d[128 * k:128 * (k + 1), xcut[0]:xcut[1]])
            cpk = const.tile([128, 512], dt.bfloat16, tag="cpk", name="cpk")
            nc.sync.dma_start(out=cpk, in_=cpack_d[:, :])
            rblk = cpk[:, 0:128]
            id16 = cpk[:, 128:256]
            hmask = cpk[:, 256:384]
            hmk = cpk[:, 384:512]
            srep = const.tile([128, T], dt.bfloat16, tag="srep", name="srep")
            crep = const.tile([128, T], dt.bfloat16, tag="crep", name="crep")
            nc.sync.dma_start(out=srep[:, 0:1024], in_=srep_d[:, 0:1024])
            nc.sync.dma_start(out=crep[:, 0:1024], in_=crep_d[:, 0:1024])
            # remaining pieces, interleaved in need order
            for p in range(1, 4):
                for k in range(2):
                    nc.sync.dma_start(
                        out=xct[k][:, xcut[p]:xcut[p + 1]],
                        in_=xct_d[128 * k:128 * (k + 1), xcut[p]:xcut[p + 1]])
                sl = slice(p * 1024, (p + 1) * 1024)
                nc.sync.dma_start(out=srep[:, sl], in_=srep_d[:, sl])
                nc.sync.dma_start(out=crep[:, sl], in_=crep_d[:, sl])
            wqop = const.tile([128, 1536], dt.bfloat16, tag="wqop", name="wqop")
            nc.sync.dma_start(out=wqop, in_=wqop_d[:, :])
            wq = [wqop[:, 256 * k:256 * (k + 1)] for k in range(2)]
            wo = [wqop[:, 512 + 256 * k:512 + 256 * (k + 1)] for k in range(2)]
            wp = [wqop[:, 1024 + 256 * k:1024 + 256 * (k + 1)] for k in range(2)]
            dcw = load("dcw", dcw_d, [128, 768])
            ones = const.tile([1, CH], dt.bfloat16, tag="ones", name="ones")
            nc.vector.memset(ones, 1.0)
            if use_bias:
                bqkvo = load("bqkvo", bqkvo_d, [1, 1024])
                blep = load("blep", blep_d, [1, 256])
                bprj = load("bprj", bprj_d, [1, 256])

            # persistent activations (paired layout: col c*1024 + j*512 + t)
            q1p = const.tile([128, 2 * T], dt.bfloat16, tag="q1p", name="q1p")
            qs0 = const.tile([128, 2 * T], dt.bfloat16, tag="qs0", name="qs0")
            o1p = const.tile([128, 2 * T], dt.bfloat16, tag="o1p", name="o1p")
            vT = [const.tile([128, TH], dt.bfloat16, tag=f"vT{j}", name=f"vT{j}")
                  for j in range(2)]
            kpart = const.tile([128, 16], dt.float32, tag="kpart", name="kpart")
            vpart = const.tile([128, 16], dt.float32, tag="vpart", name="vpart")
            stats = const.tile([128, 260], dt.float32, tag="stats", name="stats")
            stats2 = const.tile([128, 260], dt.float32, tag="stats2",
                                name="stats2")

            gram = pacc.tile([128, 256], dt.float32, tag="gram", name="gram")

            # =========================== phase K ===========================
            # k/v projections, elu(k)+1, rope(k), transposes + gram, stats.
            for c in range(NCH):
                xsl = [x[:, 1 + c * CH: 1 + (c + 1) * CH] for x in xct]
                ssl = srep[:, c * CH:(c + 1) * CH]
                csl = crep[:, c * CH:(c + 1) * CH]

                kps = ppool.tile([128, P2], dt.float32, tag="big", name="kps")
                vps = ppool.tile([128, P2], dt.float32, tag="big", name="vps")
                for j in range(2):
                    cols = slice(j * CH, (j + 1) * CH)
                    ksl = slice(128 * j, 128 * (j + 1))
                    vsl = slice(256 + 128 * j, 256 + 128 * (j + 1))
                    nc.tensor.matmul(kps[:, cols], wkv[0][:, ksl], xsl[0],
                                     start=True, stop=False)
                    nc.tensor.matmul(kps[:, cols], wkv[1][:, ksl], xsl[1],
                                     start=False, stop=not use_bias)
                    if use_bias:
                        nc.tensor.matmul(kps[:, cols],
                                         bqkvo[:, 256 + 128 * j:256 + 128 * (j + 1)],
                                         ones, start=False, stop=True)
                    nc.tensor.matmul(vps[:, cols], wkv[0][:, vsl], xsl[0],
                                     start=True, stop=False)
                    nc.tensor.matmul(vps[:, cols], wkv[1][:, vsl], xsl[1],
                                     start=False, stop=not use_bias)
                    if use_bias:
                        nc.tensor.matmul(vps[:, cols],
                                         bqkvo[:, 512 + 128 * j:512 + 128 * (j + 1)],
                                         ones, start=False, stop=True)

                # elu(k)+1 : exp(min(k,0)) = min(exp(k),1);  +relu(k)
                ek = work.tile([128, P2], dt.bfloat16, tag="ek", name="ek")
                nc.scalar.activation(ek, kps, AF.Exp)
                mnk = work.tile([128, P2], dt.bfloat16, tag="mnk", name="mnk")
                nc.vector.tensor_scalar_min(mnk, ek, 1.0)
                k1t = work.tile([128, P2], dt.bfloat16, tag="k1t", name="k1t")
                for j in range(2):
                    cols = slice(j * CH, (j + 1) * CH)
                    nc.vector.scalar_tensor_tensor(
                        out=k1t[:, cols], in0=kps[:, cols], scalar=0.0,
                        in1=mnk[:, cols], op0=OP.max, op1=OP.add,
                        accum_out=kpart[:, 8 * j + c: 8 * j + c + 1])
                    # v evac -> vT channel-major (vsum rides ACT accum)
                    nc.scalar.activation(
                        vT[j][:, 1 + c * CH: 1 + (c + 1) * CH],
                        vps[:, j * CH:(j + 1) * CH], AF.Copy,
                        accum_out=vpart[:, 8 * j + c: 8 * j + c + 1])

                # rope(k): ks = k1t*cos + (R.T@k1t)*sin
                m1 = work.tile([128, P2], dt.bfloat16, tag="m1", name="m1")
                nc.gpsimd.tensor_mul(
                    m1[:, :].rearrange("p (r t) -> p r t", r=2),
                    k1t[:, :].rearrange("p (r t) -> p r t", r=2),
                    csl.unsqueeze(1).to_broadcast((128, 2, CH)))
                m2 = work.tile([128, P2], dt.bfloat16, tag="m2", name="m2")
                for j in range(2):
                    cols = slice(j * CH, (j + 1) * CH)
                    rkp = ppool.tile([128, CH], dt.float32, tag="tp", bufs=3,
                                     name="rkp")
                    nc.tensor.matmul(rkp, rblk, k1t[:, cols],
                                     start=True, stop=True)
                    nc.vector.tensor_mul(m2[:, cols], rkp, ssl)
                ks = work.tile([128, P2], dt.bfloat16, tag="ks", name="ks")
                nc.vector.tensor_add(ks, m1, m2)

                # transposes to token-major; kv gram accumulation
                for s in range(4):
                    ktp = ppool.tile([128, CH], dt.bfloat16, tag="tp", bufs=3,
                                     name="ktp")
                    nc.tensor.transpose(ktp[:, 0:128],
                                        ks[:, s * 128:(s + 1) * 128], id16)
                    nc.tensor.transpose(ktp[:, 128:256],
                                        ks[:, CH + s * 128:CH + (s + 1) * 128],
                                        id16)
                    vcol = 1 + c * CH + s * 128
                    nc.tensor.transpose(ktp[:, 256:384],
                                        vT[0][:, vcol:vcol + 128], id16)
                    nc.tensor.transpose(ktp[:, 384:512],
                                        vT[1][:, vcol:vcol + 128], id16)
                    kvtok = work.tile([128, CH], dt.bfloat16, tag="kvtok",
                                      name="kvtok")
                    if s % 2 == 1:
                        nc.vector.tensor_copy(kvtok, ktp)
                    else:
                        nc.scalar.activation(kvtok, ktp, AF.Copy)
                    first = (c == 0 and s == 0)
                    last = (c == NCH - 1 and s == 3)
                    nc.tensor.matmul(gram[:, 0:128], kvtok[:, 0:128],
                                     kvtok[:, 256:384], start=first, stop=False)
                    nc.tensor.matmul(gram[:, 128:256], kvtok[:, 128:256],
                                     kvtok[:, 384:512], start=False, stop=last)

            # ---- halo v columns (tokens t0-1 and t0+T) for the conv
            vhp = ppool.tile([128, CH], dt.float32, tag="tp", bufs=3, name="vhp")
            for j in range(2):
                vsl = slice(256 + 128 * j, 256 + 128 * (j + 1))
                cl = slice(j * 4, j * 4 + 1)
                cr = slice(j * 4 + 2, j * 4 + 3)
                nc.tensor.matmul(vhp[:, cl], wkv[0][:, vsl], xct[0][:, 0:1],
                                 start=(j == 0), stop=False)
                nc.tensor.matmul(vhp[:, cl], wkv[1][:, vsl], xct[1][:, 0:1],
                                 start=False, stop=False)
                nc.tensor.matmul(vhp[:, cr], wkv[0][:, vsl], xct[0][:, TH - 1:TH],
                                 start=False, stop=False)
                nc.tensor.matmul(vhp[:, cr], wkv[1][:, vsl], xct[1][:, TH - 1:TH],
                                 start=False, stop=(j == 1))
            for j in range(2):
                nc.scalar.activation(vT[j][:, 0:1], vhp[:, j * 4:j * 4 + 1],
                                     AF.Copy)
                nc.scalar.activation(vT[j][:, TH - 1:TH],
                                     vhp[:, j * 4 + 2:j * 4 + 3], AF.Copy)

            # ======================= stats + allreduce =====================
            nc.vector.tensor_scalar_mul(stats[:, 0:256], gram, 1.0)
            nc.vector.tensor_reduce(stats[:, 256:257], kpart[:, 0:8],
                                    axis=mybir.AxisListType.X, op=OP.add)
            nc.vector.tensor_reduce(stats[:, 257:258], kpart[:, 8:16],
                                    axis=mybir.AxisListType.X, op=OP.add)
            nc.vector.tensor_reduce(stats[:, 258:259], vpart[:, 0:8],
                                    axis=mybir.AxisListType.X, op=OP.add)
            nc.vector.tensor_reduce(stats[:, 259:260], vpart[:, 8:16],
                                    axis=mybir.AxisListType.X, op=OP.add)

            if nocc:
                nc.vector.tensor_scalar_mul(stats2, stats, 1.0)
            else:
                ccin = dpool.tile([128, 260], dt.float32, tag="ccin",
                                  name="ccin")
                ccout = dpool.tile([128, 260], dt.float32, tag="ccout",
                                   name="ccout")
                nc.gpsimd.dma_start(out=ccin[:, :], in_=stats)
                nc.gpsimd.collective_compute(
                    "AllReduce", OP.add, replica_groups=RG,
                    ins=[ccin[:, :]], outs=[ccout[:, :]])
                # dma-back on sync so the gpsimd queue stays free during
                # phase Q (dependency on the collective is tracked via ccout)
                nc.sync.dma_start(out=stats2, in_=ccout[:, :])

            # =========================== phase Q ===========================
            # q/o projections + rope(q).  Independent of the collective ->
            # overlaps it.  No gpsimd here (its queue waits on the AR).
            for c in range(NCH):
                xsl = [x[:, 1 + c * CH: 1 + (c + 1) * CH] for x in xct]
                psl = slice(c * P2, (c + 1) * P2)
                ssl = srep[:, c * CH:(c + 1) * CH]
                csl = crep[:, c * CH:(c + 1) * CH]

                qps = ppool.tile([128, P2], dt.float32, tag="big", name="qps")
                ops_ = ppool.tile([128, P2], dt.float32, tag="big", name="ops_")
                for j in range(2):
                    cols = slice(j * CH, (j + 1) * CH)
                    msl = slice(128 * j, 128 * (j + 1))
                    nc.tensor.matmul(qps[:, cols], wq[0][:, msl], xsl[0],
                                     start=True, stop=False)
                    nc.tensor.matmul(qps[:, cols], wq[1][:, msl], xsl[1],
                                     start=False, stop=not use_bias)
                    if use_bias:
                        nc.tensor.matmul(qps[:, cols], bqkvo[:, msl], ones,
                                         start=False, stop=True)
                    nc.tensor.matmul(ops_[:, cols], wo[0][:, msl], xsl[0],
                                     start=True, stop=False)
                    nc.tensor.matmul(ops_[:, cols], wo[1][:, msl], xsl[1],
                                     start=False, stop=not use_bias)
                    if use_bias:
                        nc.tensor.matmul(ops_[:, cols],
                                         bqkvo[:, 768 + 128 * j:768 + 128 * (j + 1)],
                                         ones, start=False, stop=True)

                # elu(q)+1 -> q1p
                eq = work.tile([128, P2], dt.bfloat16, tag="ek", name="eq")
                nc.scalar.activation(eq, qps, AF.Exp)
                mnq = work.tile([128, P2], dt.bfloat16, tag="mnk", name="mnq")
                nc.vector.tensor_scalar_min(mnq, eq, 1.0)
                nc.vector.scalar_tensor_tensor(
                    out=q1p[:, psl], in0=qps, scalar=0.0, in1=mnq,
                    op0=OP.max, op1=OP.add)

                # o evac
                nc.scalar.activation(o1p[:, psl], ops_, AF.Copy)

                # rope(q) -> qs0  (all on DVE; gpsimd queue is busy w/ AR)
                m1 = work.tile([128, P2], dt.bfloat16, tag="m1", name="m1q")
                nc.vector.tensor_mul(
                    m1[:, :].rearrange("p (r t) -> p r t", r=2),
                    q1p[:, psl].rearrange("p (r t) -> p r t", r=2),
                    csl.unsqueeze(1).to_broadcast((128, 2, CH)))
                m2 = work.tile([128, P2], dt.bfloat16, tag="m2", name="m2q")
                for j in range(2):
                    cols = slice(j * CH, (j + 1) * CH)
                    rqp = ppool.tile([128, CH], dt.float32, tag="tp", bufs=3,
                                     name="rqp")
                    nc.tensor.matmul(
                        rqp, rblk,
                        q1p[:, c * P2 + j * CH: c * P2 + (j + 1) * CH],
                        start=True, stop=True)
                    nc.vector.tensor_mul(m2[:, cols], rqp, ssl)
                nc.vector.tensor_add(qs0[:, psl], m1, m2)

            # =========================== phase Z ===========================
            zsc = const.tile([128, 2], dt.float32, tag="zsc", name="zsc")
            nc.scalar.activation(zsc, stats2[:, 256:258], AF.Copy,
                                 scale=float(SCALE / N))
            zblk = []
            kvblk = []
            mcorr = []
            for j in range(2):
                zb = const.tile([128, 128], dt.bfloat16, tag=f"zblk{j}",
                                name=f"zblk{j}")
                nc.vector.tensor_tensor(
                    zb, zsc[:, j:j + 1].to_broadcast((128, 128)), hmask,
                    OP.mult)
                zblk.append(zb)
                kvb = const.tile([128, 128], dt.bfloat16, tag=f"kvb{j}",
                                 name=f"kvb{j}")
                nc.vector.tensor_tensor(
                    kvb, stats2[:, 128 * j:128 * (j + 1)], hmk, OP.mult)
                kvblk.append(kvb)

            # vmean row: cast (scaled by -1/N) -> transpose -> broadcast
            vrin = const.tile([128, 2], dt.bfloat16, tag="vrin", name="vrin")
            nc.scalar.activation(vrin, stats2[:, 258:260], AF.Copy,
                                 scale=float(-1.0 / N))
            vtp = ppool.tile([128, CH], dt.bfloat16, tag="tp", bufs=3,
                             name="vtp")
            for j in range(2):
                nc.tensor.transpose(vtp[0:1, 128 * j:128 * (j + 1)],
                                    vrin[:, j:j + 1], id16)
            vrow = const.tile([1, 256], dt.bfloat16, tag="vrow", name="vrow")
            nc.scalar.activation(vrow, vtp[0:1, 0:256], AF.Copy)
            for j in range(2):
                # broadcast vrow across partitions via a rank-1 matmul
                vrbp = ppool.tile([128, CH], dt.float32, tag="tp", bufs=3,
                                  name="vrbp")
                nc.tensor.matmul(vrbp[:, 0:128], ones[:, 0:128],
                                 vrow[:, 128 * j:128 * (j + 1)],
                                 start=True, stop=True)
                mc = const.tile([128, 128], dt.bfloat16, tag=f"mc{j}",
                                name=f"mc{j}")
                nc.vector.tensor_tensor(mc, zblk[j], vrbp[:, 0:128], OP.mult)
                mcorr.append(mc)

            # =========================== phase C ===========================
            # software-pipelined: chunk c+1's z/recip/qa (DVE) runs under
            # chunk c's y/outproj (PE/ACT).
            def zrq(c):
                rz = work.tile([128, P2], dt.float32, tag="rz", name="rz")
                qa = work.tile([128, P2], dt.bfloat16, tag="qa", name="qa")
                for j in range(2):
                    cols = slice(j * CH, (j + 1) * CH)
                    qsl = slice(c * P2 + j * CH, c * P2 + (j + 1) * CH)
                    zps = ppool.tile([128, CH], dt.float32, tag="tp", bufs=3,
                                     name="zps")
                    nc.tensor.matmul(zps, zblk[j], q1p[:, qsl],
                                     start=True, stop=True)
                    nc.vector.reciprocal_approx_fast(out=rz[:, cols], in_=zps)
                    nc.vector.scalar_tensor_tensor(
                        out=qa[:, cols], in0=rz[:, cols], scalar=1.0,
                        in1=qs0[:, qsl], op0=OP.add, op1=OP.mult)
                return qa

            qa_c = zrq(0)
            for c in range(NCH):
                psl = slice(c * P2, (c + 1) * P2)
                rps = ppool.tile([128, P2], dt.float32, tag="big", name="rps")
                for j in range(2):
                    cols = slice(j * CH, (j + 1) * CH)
                    qsl = slice(c * P2 + j * CH, c * P2 + (j + 1) * CH)
                    nc.tensor.matmul(rps[:, cols], kvblk[j], qa_c[:, cols],
                                     start=True, stop=False)
                    nc.tensor.matmul(rps[:, cols], mcorr[j], q1p[:, qsl],
                                     start=False, stop=False)
                    for tap in range(3):
                        lastmm = (tap == 2 and not use_bias)
                        nc.tensor.matmul(
                            rps[:, cols],
                            dcw[:, (tap * 2 + j) * 128:(tap * 2 + j + 1) * 128],
                            vT[j][:, c * CH + tap: c * CH + tap + CH],
                            start=False, stop=lastmm)
                    if use_bias:
                        nc.tensor.matmul(rps[:, cols],
                                         blep[:, 128 * j:128 * (j + 1)],
                                         ones, start=False, stop=True)

                if c + 1 < NCH:
                    qa_n = zrq(c + 1)

                y = work.tile([128, P2], dt.bfloat16, tag="y", name="y")
                nc.vector.tensor_mul(y, rps, o1p[:, psl])

                # output projection, channel-major: outc[oc, t] (per oc-half)
                for h in range(2):
                    hsl = slice(128 * h, 128 * (h + 1))
                    outc = ppool.tile([128, CH], dt.float32, tag="tp", bufs=3,
                                      name="outc")
                    nc.tensor.matmul(outc, wp[0][:, hsl], y[:, 0:CH],
                                     start=True, stop=False)
                    nc.tensor.matmul(outc, wp[1][:, hsl], y[:, CH:P2],
                                     start=False, stop=not use_bias)
                    if use_bias:
                        nc.tensor.matmul(outc, bprj[:, hsl], ones,
                                         start=False, stop=True)
                    outsb = work.tile([128, CH], dt.bfloat16, tag="outsb",
                                      name="outsb")
                    nc.scalar.activation(outsb, outc, AF.Copy)
                    nc.sync.dma_start(
                        out=out_d[128 * h:128 * (h + 1), c * CH:(c + 1) * CH],
                        in_=outsb)
                if c + 1 < NCH:
                    qa_c = qa_n

    nc.compile()
    return nc


_NC_CACHE = {}


def _get_nc(use_bias: bool):
    nocc = bool(os.environ.get("KERNEL_NOCC"))
    key = (use_bias, nocc)
    if key not in _NC_CACHE:
        _NC_CACHE[key] = _build_nc(use_bias, nocc)
    return _NC_CACHE[key]


def kernel(x, sin, cos, W_qkvo, b_qkvo, W_lepe, b_lepe, W_proj, b_proj):
    from concourse.bass_utils import run_bass_kernel_spmd

    per_core, use_bias = _host_prep(x, sin, cos, W_qkvo, b_qkvo, W_lepe,
                                    b_lepe, W_proj, b_proj)
    nc = _get_nc(use_bias)
    # keep only the inputs that survived DCE in the compiled program
    import concourse.mybir as mybir
    expected = set()
    for alloc in nc.m.functions[0].allocations:
        if isinstance(alloc, mybir.MemoryLocationSet) and alloc.kind == "ExternalInput":
            expected.add(alloc.memorylocations[0].name)
    per_core = [{k: v for k, v in m.items() if k in expected} for m in per_core]
    res = run_bass_kernel_spmd(nc, per_core, core_ids=list(range(NCORES)),
                               trace=bool(os.environ.get("KERNEL_TRACE")))
    if os.environ.get("KERNEL_TRACE"):
        kernel.last_exec_time_ns = res.exec_time_ns
        kernel.last_results = res
    full = np.zeros((B, N, INTERNAL), np.float32)
    for c in range(NCORES):
        b = c // 2
        t0 = (c % 2) * T
        full[b, t0:t0 + T] = res.results[c]["out"].astype(np.float32).T
    return full


# ---------------------------------------------------------- numpy reference
# A numpy emulation of the device pipeline (fp32), used to validate the
# decomposition (run with KERNEL_SELFTEST=1).

def _numpy_pipeline(per_core_inputs, skip_pair=False):
    outs = []
    cores = []
    for c in range(NCORES):
        d = per_core_inputs[c]
        xct = d["xct"].astype(np.float32)          # [256, TH]
        srep = d["srep"].astype(np.float32)
        crep = d["crep"].astype(np.float32)
        wqop = d["wqop"].astype(np.float32)
        wq = np.concatenate([wqop[:, 0:256], wqop[:, 256:512]], axis=0)
        wo = np.concatenate([wqop[:, 512:768], wqop[:, 768:1024]], axis=0)
        wp = np.concatenate([wqop[:, 1024:1280], wqop[:, 1280:1536]], axis=0)
        wkv = d["wkv"].astype(np.float32)
        dcw = d["dcw"].astype(np.float32).reshape(128, 6, 128)
        cpack = d["cpack"].astype(np.float32)
        R = cpack[:, 0:128]
        hmask = cpack[:, 256:384]

        x_in = xct[:, 1:T + 1]                     # [256, T]
        qT = wq.T @ x_in                           # [256, T]
        kT = wkv[:, 0:256].T @ x_in
        vT_m = wkv[:, 256:512].T @ x_in
        oT = wo.T @ x_in
        # halo v cols
        vhl = wkv[:, 256:512].T @ xct[:, 0:1]
        vhr = wkv[:, 256:512].T @ xct[:, TH - 1:TH]
        vT = np.concatenate([vhl, vT_m, vhr], axis=1)      # [256, TH]

        def elu1(t):
            return np.minimum(np.exp(t), 1.0) + np.maximum(t, 0.0)

        q1 = elu1(qT)
        k1 = elu1(kT)

        # K rope (per chan-tile with R)
        ks = np.zeros_like(k1)
        qs = np.zeros_like(q1)
        for j in range(2):
            blk = k1[128 * j:128 * (j + 1)]
            ks[128 * j:128 * (j + 1)] = blk * crep + (R.T @ blk) * srep
            qb = q1[128 * j:128 * (j + 1)]
            qs[128 * j:128 * (j + 1)] = qb * crep + (R.T @ qb) * srep

        # kv gram per tile: ks_j^T tokens x v_j
        gram = np.zeros((128, 256), np.float32)
        for j in range(2):
            gram[:, 128 * j:128 * (j + 1)] = (
                ks[128 * j:128 * (j + 1)] @ vT[128 * j:128 * (j + 1), 1:T + 1].T)
        ksum = k1.sum(axis=1)                      # [256]
        vsum = vT[:, 1:T + 1].sum(axis=1)
        cores.append(dict(d=d, q1=q1, qs=qs, oT=oT, vT=vT, gram=gram,
                          ksum=ksum, vsum=vsum, R=R, hmask=hmask, dcw=dcw,
                          wp=wp))

    for pair in range(4):
        a, b2 = cores[2 * pair], cores[2 * pair + 1]
        if skip_pair:
            for cc in (a, b2):
                cc["gram_r"], cc["ksum_r"], cc["vsum_r"] = (
                    cc["gram"], cc["ksum"], cc["vsum"])
            continue
        gram = a["gram"] + b2["gram"]
        ksum = a["ksum"] + b2["ksum"]
        vsum = a["vsum"] + b2["vsum"]
        for cc in (a, b2):
            cc["gram_r"], cc["ksum_r"], cc["vsum_r"] = gram, ksum, vsum

    for c in range(NCORES):
        st = cores[c]
        q1, qs, oT, vT = st["q1"], st["qs"], st["oT"], st["vT"]
        hmask, dcw, wp = st["hmask"], st["dcw"], st["wp"]
        gram, ksum, vsum = st["gram_r"], st["ksum_r"], st["vsum_r"]

        kmean = ksum / N
        vmean = vsum / N
        res = np.zeros((256, T), np.float32)
        for j in range(2):
            sl = slice(128 * j, 128 * (j + 1))
            zsc = SCALE * kmean[sl]                          # [128]
            zblk = (zsc[:, None] * hmask)                    # [128,128]
            zrep = zblk.T @ q1[sl]                           # [128, T]
            r = 1.0 / zrep
            qa = qs[sl] * (1.0 + r)
            kvblk = KSC * gram[:, 128 * j:128 * (j + 1)] * hmask
            mcorr = -(zsc[:, None]) * vmean[sl][None, :] * hmask
            lepe = np.zeros((128, T), np.float32)
            for tap in range(3):
                dw = dcw[:, tap * 2 + j, :]
                lepe += dw.T @ vT[sl, tap:tap + T]
            res[sl] = (kvblk.T @ qa + mcorr.T @ q1[sl] + lepe)
        y = res * oT
        out = y.T @ wp
        outs.append(out.astype(np.float32))

    # unshard
    full = np.zeros((B, N, 256), np.float32)
    for c in range(NCORES):
        b = c // 2
        t0 = (c % 2) * T
        full[b, t0:t0 + T] = outs[c]
    return full


if __name__ == "__main__" and os.environ.get("KERNEL_BUILD"):
    nc = _build_nc(False)
    import tempfile
    from concourse.bass_utils import compile_bass_kernel
    print("NEFF:", compile_bass_kernel(nc, tempfile.mkdtemp()))

if __name__ == "__main__" and os.environ.get("KERNEL_SELFTEST"):
    sys.path.insert(0, os.path.dirname(os.path.abspath(__file__)))
    import reference
    inputs = reference.setup_inputs()
    inputs = {k: np.asarray(v) for k, v in inputs.items()}
    expected = np.asarray(reference.reference(**inputs))
    per_core, use_bias = _host_prep(**inputs)
    got = _numpy_pipeline(per_core)
    err = np.abs(got - expected)
    rel = np.linalg.norm(got - expected) / np.linalg.norm(expected)
    print("selftest rel err:", rel, "max abs:", err.max())
